# revision 8
# baseline (speedup 1.0000x reference)
"""Bass/Trainium2 kernel for nn_DWAMiddleLayer (low-rank MoE weight-assembly layer).

Math (reference):
    U    = pool[:, :1024].reshape(N, DB, R)      # [512, 256, 4]
    V    = pool[:, 1024:2048].reshape(N, R, DA)  # [512, 4, 256]
    bE   = pool[:, 2048:2304]                    # [512, 256]
    h_t  = h_A @ W_base.T
           + sum_r (alpha * (h_A @ V_r.T)) @ U_r          # never materialize W_assembled
           + alpha @ bE + b_base
    y    = h_A + gamma * h_t ; out = LayerNorm(y) * ln_scale + ln_bias

Distribution: data-parallel over batch B=2048 across 8 cores (BS=256 rows each);
pool/W_base/vectors replicated.

v3: all matmul operands pre-transposed/pre-cast to bf16 on the host; pool HBM
layout gives >=9.2KB contiguous per partition per DMA (HW DMA queues are
latency-bound on smaller descriptors); DMAs ordered by first use on one HWDGE
ring; PE warmed with dummy matmuls so the HAM clock gate lifts before real
work; alpha*t as a single direct-from-PSUM DVE multiply per expert chunk;
LN epilogue normalize on the Scalar engine (per-partition scale/bias), with
per-batch-chunk output DMAs on both HWDGE rings.
"""

import numpy as np

B, N, D_A, D_B, R = 2048, 512, 256, 256, 4
NC_COUNT = 8
BS = B // NC_COUNT  # 256 batch rows per core
P = 128
LN_EPS = 1e-5

# ---- packed small tensor A (bf16 cols), needed early ----
SA_HAT = 0      # hA^T      [p_a, 2 ach, 256 b]
SA_ALT = 512    # alpha^T   [p_n, 4 och, 256 b]
SA_ID = 1536    # ident     [p, 128] bf16
SA_WBT = 1664   # W_base^T  [p_a, 2 ach, 256 c]
SA_GE = 2176    # fp32 [gamma, eps] bitcast -> 4 bf16 cols
SA_W = 2180
# ---- packed small tensor B (bf16 cols), needed late (epilogue) ----
SB_HAF = 0      # h_A fp32  [p_b, 2 bch, 256 a] bitcast -> 1024 bf16 cols
SB_LSC = 1024   # ln_scale  [p, 256] replicated
SB_LBI = 1280   # ln_bias   [p, 256] replicated
SB_BB = 1536    # b_base    row0 only [1, 256]
SB_W = 1792
# ---- pool chunk layout (per o = n//128): [VT 1024 | U2 1024 | bE 256] ----
PO_VT = 0       # [ach(2), r(4), pn(128)]
PO_U2 = 1024    # [r(4), cch(2), pc(128)]
PO_BE = 2048    # [c(256)]
PO_W = 2304

N_WARM = 10  # warm-up matmuls (j=512): ~4.3us of cold PE activity to lift HAM

_cache = {}


def _build_nc():
    import concourse.mybir as mybir
    import concourse.tile as tile
    from concourse import bacc

    fp32 = mybir.dt.float32
    bf16 = mybir.dt.bfloat16

    nc = bacc.Bacc("TRN2", target_bir_lowering=False)

    # ---- DRAM I/O (per-core shard shapes) ----
    d_sa = nc.dram_tensor("sma", [P, SA_W], bf16, kind="ExternalInput")
    d_pool = nc.dram_tensor("pool", [P, 4, PO_W], bf16, kind="ExternalInput")
    d_sb = nc.dram_tensor("smb", [P, SB_W], bf16, kind="ExternalInput")
    d_out = nc.dram_tensor("out", [BS, D_A], fp32, kind="ExternalOutput")

    with tile.TileContext(nc) as tc:
        with (
            tc.tile_pool(name="persist", bufs=1) as persist,
            tc.tile_pool(name="stage", bufs=2) as stage,
            tc.tile_pool(name="sm", bufs=3) as sm,
            tc.tile_pool(name="pp_t", bufs=2, space="PSUM") as pp_t,
            tc.tile_pool(name="pp_acc", bufs=1, space="PSUM") as pp_acc,
            tc.tile_pool(name="pp_tr", bufs=1, space="PSUM") as pp_tr,
            tc.tile_pool(name="pp_w", bufs=1, space="PSUM") as pp_w,
        ):
            # ---------- PE warm-up: junk matmuls to lift the HAM clock gate ----------
            wsrc = persist.tile([P, 512], bf16)
            nc.vector.memset(wsrc, 0.0)
            ones_row = persist.tile([1, BS], bf16)
            nc.vector.memset(ones_row, 1.0)
            neg1_col = persist.tile([P, 1], fp32)
            nc.vector.memset(neg1_col, -1.0)
            warm_ps = pp_w.tile([P, 512], fp32, tag="warm")
            for _ in range(N_WARM):
                nc.tensor.matmul(
                    warm_ps, lhsT=wsrc[:, 0:P], rhs=wsrc, start=True, stop=True,
                    skip_group_check=True,
                )

            # ---------- loads (HWDGE, ordered by first use) ----------
            sa = persist.tile([P, SA_W], bf16)
            nc.sync.dma_start(sa, d_sa[:])
            pool01 = stage.tile([P, 2, PO_W], bf16, tag="pool", name="pool01")
            nc.sync.dma_start(pool01, d_pool[:, 0:2])
            pool23 = stage.tile([P, 2, PO_W], bf16, tag="pool", name="pool23")
            nc.sync.dma_start(pool23, d_pool[:, 2:4])
            sb = persist.tile([P, SB_W], bf16)
            nc.sync.dma_start(sb, d_sb[:])

            hAT = sa[:, SA_HAT : SA_HAT + 512].rearrange("p (a b) -> p a b", a=2)
            alphaT = sa[:, SA_ALT : SA_ALT + 1024].rearrange("p (o b) -> p o b", o=4)
            ident_b = sa[:, SA_ID : SA_ID + P]
            WbT = sa[:, SA_WBT : SA_WBT + 512].rearrange("p (a c) -> p a c", a=2)
            ge = sa[:, SA_GE : SA_GE + 4].bitcast(fp32)
            gamma_col = ge[:, 0:1]
            eps_col = ge[:, 1:2]
            hA_f32 = sb[:, SB_HAF : SB_HAF + 1024].bitcast(fp32).rearrange(
                "p (o a) -> p o a", o=2
            )
            lsc_row = sb[:, SB_LSC : SB_LSC + 256]
            lbi_row = sb[:, SB_LBI : SB_LBI + 256]
            bb_row = sb[0:1, SB_BB : SB_BB + 256]

            # warm the ACT tables (Copy for the copies, Sqrt for the LN tail)
            warm_act = sm.tile([P, 1], fp32, tag="warmact")
            nc.scalar.activation(
                warm_act, wsrc[:, 0:1], mybir.ActivationFunctionType.Copy
            )
            nc.scalar.activation(
                warm_act, wsrc[:, 0:1], mybir.ActivationFunctionType.Sqrt
            )

            # ---------- h_t^T accumulator: one psum tile [p_c-half, cch, b] ----------
            h_acc = pp_acc.tile([P, 2, BS], fp32, tag="acc")
            started = [False, False]

            def acc_mm(ch, lhsT, rhs, last=False):
                nc.tensor.matmul(
                    h_acc[:, ch],
                    lhsT=lhsT,
                    rhs=rhs,
                    start=(not started[ch]),
                    stop=last,
                    skip_group_check=True,
                )
                started[ch] = True

            # ---------- main pipeline over expert chunks (o = n//128) ----------
            for o in range(4):
                pt = pool01[:, o] if o < 2 else pool23[:, o - 2]
                VT_o = pt[:, PO_VT : PO_VT + 1024].rearrange(
                    "p (a r q) -> p a r q", a=2, r=4
                )
                U2_o = pt[:, PO_U2 : PO_U2 + 1024].rearrange(
                    "p (r c q) -> p r c q", r=4, c=2
                )
                bE_o = pt[:, PO_BE : PO_BE + 256]

                # mm1: t^T[(n), r, b] = sum_a V[n,r,a] * hA[b,a]
                t_ps = pp_t.tile([P, 4, BS], fp32, tag="t")
                for r in range(4):
                    for ach in range(2):
                        nc.tensor.matmul(
                            t_ps[:, r],
                            lhsT=VT_o[:, ach, r],
                            rhs=hAT[:, ach],
                            start=(ach == 0),
                            stop=(ach == 1),
                        )
                # s = alpha * t : one direct-from-PSUM DVE multiply per chunk
                s_bf = sm.tile([P, 4, BS], bf16, tag="sbf")
                nc.vector.tensor_mul(
                    s_bf, t_ps, alphaT[:, o : o + 1, :].to_broadcast((P, 4, BS))
                )
                # mm2: h_t^T += U_r^T-chunks @ s_r^T (contract n)
                for r in range(4):
                    for ch in range(2):
                        acc_mm(ch, U2_o[:, r, ch], s_bf[:, r])
                # bias-mm: h_t^T += biasE^T @ alpha^T (contract n)
                for ch in range(2):
                    acc_mm(ch, bE_o[:, ch * P : (ch + 1) * P], alphaT[:, o])

                if o == 1:
                    # base-mm (W_base^T arrives with sa; fold in mid-stream)
                    for ch in range(2):
                        for ach in range(2):
                            acc_mm(ch, WbT[:, ach, ch * P : (ch + 1) * P], hAT[:, ach])
            # b_base rank-1 last (needs sb)
            for ch in range(2):
                acc_mm(ch, bb_row[:, ch * P : (ch + 1) * P], ones_row, last=True)

            # ---------- epilogue: transpose h_t back, residual + LayerNorm ----------
            ht_bf = sm.tile([P, 2, BS], bf16, tag="htbf")
            nc.scalar.activation(ht_bf, h_acc, mybir.ActivationFunctionType.Copy)

            ht_ps = pp_tr.tile([P, 2, D_A], fp32, tag="tr")
            for bch in range(2):
                for cch in range(2):
                    nc.tensor.matmul(
                        ht_ps[:, bch, cch * P : (cch + 1) * P],
                        lhsT=ht_bf[:, cch, bch * P : (bch + 1) * P],
                        rhs=ident_b,
                        start=True,
                        stop=True,
                        skip_group_check=True,
                    )

            # y = h_A + gamma * h_t (fp32 residual), both b-chunks in one pass
            y_sb = sm.tile([P, 2, D_A], fp32, tag="y")
            nc.vector.scalar_tensor_tensor(
                y_sb,
                in0=ht_ps,
                scalar=gamma_col,
                in1=hA_f32,
                op0=mybir.AluOpType.mult,
                op1=mybir.AluOpType.add,
            )
            stats = sm.tile([P, 2, 6], fp32, tag="st")
            mv = sm.tile([P, 2, 2], fp32, tag="mv")
            for bch in range(2):
                nc.vector.bn_stats(stats[:, bch], y_sb[:, bch])
                nc.vector.bn_aggr(mv[:, bch], stats[:, bch])
            # rstd = 1/sqrt(var + eps) for both chunks at once
            rstd = sm.tile([P, 2], fp32, tag="rstd")
            nc.scalar.activation(
                rstd,
                mv[:, :, 1],
                mybir.ActivationFunctionType.Sqrt,
                bias=eps_col,
            )
            nc.vector.reciprocal(rstd, rstd)
            # nmr = -mu * rstd (per-partition bias for the ACT normalize)
            nmr = sm.tile([P, 2], fp32, tag="nmr")
            nc.vector.scalar_tensor_tensor(
                nmr,
                in0=mv[:, :, 0],
                scalar=neg1_col,
                in1=rstd,
                op0=mybir.AluOpType.mult,
                op1=mybir.AluOpType.mult,
            )
            # per-batch-chunk: normalize on ACT, scale/bias on DVE, DMA out
            w_sb = sm.tile([P, 2, D_A], fp32, tag="w")
            out_sb = sm.tile([P, 2, D_A], fp32, tag="out")
            for bch in range(2):
                nc.scalar.activation(
                    w_sb[:, bch],
                    y_sb[:, bch],
                    mybir.ActivationFunctionType.Identity,
                    bias=nmr[:, bch : bch + 1],
                    scale=rstd[:, bch : bch + 1],
                )
                nc.vector.tensor_mul(w_sb[:, bch], w_sb[:, bch], lsc_row)
                nc.vector.tensor_add(out_sb[:, bch], w_sb[:, bch], lbi_row)
                eng = nc.sync if bch == 0 else nc.scalar
                eng.dma_start(d_out[bch * P : (bch + 1) * P, :], out_sb[:, bch])

    nc.compile()
    return nc


def _get_nc():
    if "nc" not in _cache:
        _cache["nc"] = _build_nc()
    return _cache["nc"]


def make_in_maps(**inputs):
    """Shard + pre-transpose + pre-cast full inputs into 8 per-core input maps."""
    import ml_dtypes

    bf = ml_dtypes.bfloat16
    f32 = lambda x: np.ascontiguousarray(np.asarray(x), dtype=np.float32)
    h_A = f32(inputs["h_A"])
    alpha = f32(inputs["alpha"])
    pool = np.asarray(inputs["pool_vectors"], dtype=np.float32)
    W_base = f32(inputs["W_base"])
    b_base = f32(inputs["b_base"]).reshape(D_B)
    gamma = float(np.asarray(inputs["gamma"]).reshape(()))
    ln_scale = f32(inputs["ln_scale"]).reshape(D_A)
    ln_bias = f32(inputs["ln_bias"]).reshape(D_A)

    U = pool[:, : D_B * R].reshape(N, D_B, R)
    V = pool[:, D_B * R : D_B * R + R * D_A].reshape(N, R, D_A)
    bE = pool[:, D_B * R + R * D_A : D_B * R + R * D_A + D_B]

    # pool chunks in final SBUF layout, bf16, o-interleaved per partition row
    pool_pk = np.empty((P, 4, PO_W), bf)
    for o in range(4):
        nsl = slice(o * P, (o + 1) * P)
        # VT[p_a, ach, r, pn] = V[o*128+pn, r, ach*128+p_a]
        vt = V[nsl].transpose(2, 1, 0).reshape(2, P, R, P).transpose(1, 0, 2, 3)
        pool_pk[:, o, PO_VT : PO_VT + 1024] = vt.reshape(P, 1024).astype(bf)
        # U2[p_n, r, cch, pc] = U[o*128+p_n, cch*128+pc, r]
        u2 = U[nsl].transpose(0, 2, 1).reshape(P, R, 2, P)
        pool_pk[:, o, PO_U2 : PO_U2 + 1024] = u2.reshape(P, 1024).astype(bf)
        pool_pk[:, o, PO_BE : PO_BE + 256] = bE[nsl].astype(bf)

    ident = np.eye(P, dtype=np.float32).astype(bf)
    ge = np.empty((P, 2), np.float32)
    ge[:, 0] = gamma
    ge[:, 1] = LN_EPS
    wbt = np.ascontiguousarray(
        W_base.T.reshape(2, P, D_B).transpose(1, 0, 2).reshape(P, 512)
    ).astype(bf)

    in_maps = []
    for i in range(NC_COUNT):
        sl = slice(i * BS, (i + 1) * BS)
        sa = np.zeros((P, SA_W), bf)
        hat = h_A[sl].T.reshape(2, P, BS).transpose(1, 0, 2).reshape(P, 512)
        sa[:, SA_HAT : SA_HAT + 512] = hat.astype(bf)
        alt = alpha[sl].T.reshape(4, P, BS).transpose(1, 0, 2).reshape(P, 1024)
        sa[:, SA_ALT : SA_ALT + 1024] = alt.astype(bf)
        sa[:, SA_ID : SA_ID + P] = ident
        sa[:, SA_WBT : SA_WBT + 512] = wbt
        sa[:, SA_GE : SA_GE + 4] = ge.view(bf)

        sb = np.zeros((P, SB_W), bf)
        haf = np.ascontiguousarray(
            h_A[sl].reshape(2, P, D_A).transpose(1, 0, 2).reshape(P, 512)
        )
        sb[:, SB_HAF : SB_HAF + 1024] = haf.view(bf)
        sb[:, SB_LSC : SB_LSC + 256] = ln_scale.astype(bf)[None, :]
        sb[:, SB_LBI : SB_LBI + 256] = ln_bias.astype(bf)[None, :]
        sb[0, SB_BB : SB_BB + 256] = b_base.astype(bf)

        in_maps.append({"sma": sa, "pool": pool_pk, "smb": sb})
    return in_maps


def run_kernel(trace=False, **inputs):
    from concourse.bass_utils import run_bass_kernel_spmd

    nc = _get_nc()
    in_maps = make_in_maps(**inputs)
    res = run_bass_kernel_spmd(nc, in_maps, core_ids=list(range(NC_COUNT)), trace=trace)
    out = np.concatenate([r["out"] for r in res.results], axis=0)
    return out.astype(np.float32), res


def kernel(**inputs) -> np.ndarray:
    out, _ = run_kernel(trace=False, **inputs)
    return out


# revision 12
# speedup vs baseline: 1.0513x; 1.0513x over previous
"""Bass/Trainium2 kernel for nn_DWAMiddleLayer (low-rank MoE weight-assembly layer).

Math (reference):
    U    = pool[:, :1024].reshape(N, DB, R)      # [512, 256, 4]
    V    = pool[:, 1024:2048].reshape(N, R, DA)  # [512, 4, 256]
    bE   = pool[:, 2048:2304]                    # [512, 256]
    h_t  = h_A @ W_base.T
           + sum_r (alpha * (h_A @ V_r.T)) @ U_r          # never materialize W_assembled
           + alpha @ bE + b_base
    y    = h_A + gamma * h_t ; out = LayerNorm(y) * ln_scale + ln_bias

Distribution: data-parallel over batch B=2048 across 8 cores (BS=256 rows each);
pool/W_base/vectors replicated.

v3: all matmul operands pre-transposed/pre-cast to bf16 on the host; pool HBM
layout gives >=9.2KB contiguous per partition per DMA (HW DMA queues are
latency-bound on smaller descriptors); DMAs ordered by first use on one HWDGE
ring; PE warmed with dummy matmuls so the HAM clock gate lifts before real
work; alpha*t as a single direct-from-PSUM DVE multiply per expert chunk;
LN epilogue normalize on the Scalar engine (per-partition scale/bias), with
per-batch-chunk output DMAs on both HWDGE rings.
"""

import numpy as np

B, N, D_A, D_B, R = 2048, 512, 256, 256, 4
NC_COUNT = 8
BS = B // NC_COUNT  # 256 batch rows per core
P = 128
LN_EPS = 1e-5

# ---- packed small tensor A (bf16 cols), needed early ----
SA_HAT = 0      # hA^T      [p_a, 2 ach, 256 b]
SA_ALT = 512    # alpha^T   [p_n, 4 och, 256 b]
SA_ID = 1536    # ident     [p, 128] bf16
SA_WBT = 1664   # W_base^T  [p_a, 2 ach, 256 c]
SA_GE = 2176    # fp32 [gamma, eps] bitcast -> 4 bf16 cols
SA_W = 2180
# ---- packed small tensor B (bf16 cols), needed late (epilogue) ----
SB_HAF = 0      # h_A fp32  [p_b, 2 bch, 256 a] bitcast -> 1024 bf16 cols
SB_LSC = 1024   # ln_scale  [p, 256] replicated
SB_LBI = 1280   # ln_bias   [p, 256] replicated
SB_BB = 1536    # b_base    row0 only [1, 256]
SB_W = 1792
# ---- pool chunk layout (per o = n//128): [VT 1024 | U2 1024 | bE 256] ----
PO_VT = 0       # [ach(2), r(4), pn(128)]
PO_U2 = 1024    # [r(4), cch(2), pc(128)]
PO_BE = 2048    # [c(256)]
PO_W = 2304

N_WARM = 7  # warm-up matmuls (j=512): ~3us of cold PE activity to lift HAM

_cache = {}


def _build_nc():
    import concourse.mybir as mybir
    import concourse.tile as tile
    from concourse import bacc

    fp32 = mybir.dt.float32
    bf16 = mybir.dt.bfloat16

    nc = bacc.Bacc("TRN2", target_bir_lowering=False)

    # ---- DRAM I/O (per-core shard shapes) ----
    d_sa = nc.dram_tensor("sma", [P, SA_W], bf16, kind="ExternalInput")
    d_pool = nc.dram_tensor("pool", [P, 4, PO_W], bf16, kind="ExternalInput")
    d_sb = nc.dram_tensor("smb", [P, SB_W], bf16, kind="ExternalInput")
    d_out = nc.dram_tensor("out", [BS, D_A], fp32, kind="ExternalOutput")

    with tile.TileContext(nc) as tc:
        with (
            tc.tile_pool(name="persist", bufs=1) as persist,
            tc.tile_pool(name="stage", bufs=4) as stage,
            tc.tile_pool(name="sm", bufs=3) as sm,
            tc.tile_pool(name="pp_t", bufs=2, space="PSUM") as pp_t,
            tc.tile_pool(name="pp_acc", bufs=1, space="PSUM") as pp_acc,
            tc.tile_pool(name="pp_tr", bufs=1, space="PSUM") as pp_tr,
            tc.tile_pool(name="pp_w", bufs=1, space="PSUM") as pp_w,
        ):
            # ---------- PE warm-up: junk matmuls to lift the HAM clock gate ----------
            wsrc = persist.tile([P, 512], bf16)
            nc.vector.memset(wsrc, 0.0)
            ones_row = persist.tile([1, BS], bf16)
            nc.vector.memset(ones_row, 1.0)
            neg1_col = persist.tile([P, 1], fp32)
            nc.vector.memset(neg1_col, -1.0)
            warm_ps = pp_w.tile([P, 512], fp32, tag="warm")
            for _ in range(N_WARM):
                nc.tensor.matmul(
                    warm_ps, lhsT=wsrc[:, 0:P], rhs=wsrc, start=True, stop=True,
                    skip_group_check=True,
                )

            # ---------- loads ----------
            # sa on the sync HWDGE ring; bulk pool + sb on SWDGE (gpsimd) whose
            # 16-lane descriptor generation sustains ~340GB/s (one HWDGE ring
            # paces out at ~205GB/s). SW queues drain FIFO, so pool chunks
            # arrive in issue order ahead of sb.
            sa = persist.tile([P, SA_W], bf16)
            nc.sync.dma_start(sa, d_sa[:])
            pool_t = [
                stage.tile([P, PO_W], bf16, tag="pool", name=f"pool{o}")
                for o in range(4)
            ]
            for o in range(4):
                nc.gpsimd.dma_start(pool_t[o], d_pool[:, o])
            sb = persist.tile([P, SB_W], bf16)
            nc.gpsimd.dma_start(sb, d_sb[:])

            hAT = sa[:, SA_HAT : SA_HAT + 512].rearrange("p (a b) -> p a b", a=2)
            alphaT = sa[:, SA_ALT : SA_ALT + 1024].rearrange("p (o b) -> p o b", o=4)
            ident_b = sa[:, SA_ID : SA_ID + P]
            WbT = sa[:, SA_WBT : SA_WBT + 512].rearrange("p (a c) -> p a c", a=2)
            ge = sa[:, SA_GE : SA_GE + 4].bitcast(fp32)
            gamma_col = ge[:, 0:1]
            eps_col = ge[:, 1:2]
            hA_f32 = sb[:, SB_HAF : SB_HAF + 1024].bitcast(fp32).rearrange(
                "p (o a) -> p o a", o=2
            )
            lsc_row = sb[:, SB_LSC : SB_LSC + 256]
            lbi_row = sb[:, SB_LBI : SB_LBI + 256]
            bb_row = sb[0:1, SB_BB : SB_BB + 256]

            # warm the ACT tables (Copy for the copies, Sqrt for the LN tail)
            warm_act = sm.tile([P, 1], fp32, tag="warmact")
            nc.scalar.activation(
                warm_act, wsrc[:, 0:1], mybir.ActivationFunctionType.Copy
            )
            nc.scalar.activation(
                warm_act, wsrc[:, 0:1], mybir.ActivationFunctionType.Sqrt
            )

            # ---------- h_t^T accumulator: one psum tile [p_c-half, cch, b] ----------
            h_acc = pp_acc.tile([P, 2, BS], fp32, tag="acc")
            started = [False, False]

            def acc_mm(ch, lhsT, rhs, last=False):
                nc.tensor.matmul(
                    h_acc[:, ch],
                    lhsT=lhsT,
                    rhs=rhs,
                    start=(not started[ch]),
                    stop=last,
                    skip_group_check=True,
                )
                started[ch] = True

            # ---------- main pipeline over expert chunks (o = n//128) ----------
            for o in range(4):
                pt = pool_t[o]
                VT_o = pt[:, PO_VT : PO_VT + 1024].rearrange(
                    "p (a r q) -> p a r q", a=2, r=4
                )
                U2_o = pt[:, PO_U2 : PO_U2 + 1024].rearrange(
                    "p (r c q) -> p r c q", r=4, c=2
                )
                bE_o = pt[:, PO_BE : PO_BE + 256]

                # mm1: t^T[(n), r, b] = sum_a V[n,r,a] * hA[b,a]
                t_ps = pp_t.tile([P, 4, BS], fp32, tag="t")
                for r in range(4):
                    for ach in range(2):
                        nc.tensor.matmul(
                            t_ps[:, r],
                            lhsT=VT_o[:, ach, r],
                            rhs=hAT[:, ach],
                            start=(ach == 0),
                            stop=(ach == 1),
                        )
                # s = alpha * t : one direct-from-PSUM DVE multiply per chunk
                s_bf = sm.tile([P, 4, BS], bf16, tag="sbf")
                nc.vector.tensor_mul(
                    s_bf, t_ps, alphaT[:, o : o + 1, :].to_broadcast((P, 4, BS))
                )
                # mm2: h_t^T += U_r^T-chunks @ s_r^T (contract n)
                for r in range(4):
                    for ch in range(2):
                        acc_mm(ch, U2_o[:, r, ch], s_bf[:, r])
                # bias-mm: h_t^T += biasE^T @ alpha^T (contract n)
                for ch in range(2):
                    acc_mm(ch, bE_o[:, ch * P : (ch + 1) * P], alphaT[:, o])

                if o == 1:
                    # base-mm (W_base^T arrives with sa; fold in mid-stream)
                    for ch in range(2):
                        for ach in range(2):
                            acc_mm(ch, WbT[:, ach, ch * P : (ch + 1) * P], hAT[:, ach])
            # b_base rank-1 last (needs sb)
            for ch in range(2):
                acc_mm(ch, bb_row[:, ch * P : (ch + 1) * P], ones_row, last=True)

            # ---------- epilogue: transpose h_t back, residual + LayerNorm ----------
            ht_bf = sm.tile([P, 2, BS], bf16, tag="htbf")
            nc.scalar.activation(ht_bf, h_acc, mybir.ActivationFunctionType.Copy)

            ht_ps = pp_tr.tile([P, 2, D_A], fp32, tag="tr")
            for bch in range(2):
                for cch in range(2):
                    nc.tensor.matmul(
                        ht_ps[:, bch, cch * P : (cch + 1) * P],
                        lhsT=ht_bf[:, cch, bch * P : (bch + 1) * P],
                        rhs=ident_b,
                        start=True,
                        stop=True,
                        skip_group_check=True,
                    )

            # y = h_A + gamma * h_t (fp32 residual), both b-chunks in one pass
            y_sb = sm.tile([P, 2, D_A], fp32, tag="y")
            nc.vector.scalar_tensor_tensor(
                y_sb,
                in0=ht_ps,
                scalar=gamma_col,
                in1=hA_f32,
                op0=mybir.AluOpType.mult,
                op1=mybir.AluOpType.add,
            )
            stats = sm.tile([P, 2, 6], fp32, tag="st")
            mv = sm.tile([P, 2, 2], fp32, tag="mv")
            for bch in range(2):
                nc.vector.bn_stats(stats[:, bch], y_sb[:, bch])
                nc.vector.bn_aggr(mv[:, bch], stats[:, bch])
            # rstd = 1/sqrt(var + eps) for both chunks at once
            rstd = sm.tile([P, 2], fp32, tag="rstd")
            nc.scalar.activation(
                rstd,
                mv[:, :, 1],
                mybir.ActivationFunctionType.Sqrt,
                bias=eps_col,
            )
            nc.vector.reciprocal(rstd, rstd)
            # nmr = -mu * rstd (per-partition bias for the ACT normalize)
            nmr = sm.tile([P, 2], fp32, tag="nmr")
            nc.vector.scalar_tensor_tensor(
                nmr,
                in0=mv[:, :, 0],
                scalar=neg1_col,
                in1=rstd,
                op0=mybir.AluOpType.mult,
                op1=mybir.AluOpType.mult,
            )
            # per-batch-chunk: normalize on ACT, scale/bias on DVE, DMA out
            w_sb = sm.tile([P, 2, D_A], fp32, tag="w")
            out_sb = sm.tile([P, 2, D_A], fp32, tag="out")
            for bch in range(2):
                nc.scalar.activation(
                    w_sb[:, bch],
                    y_sb[:, bch],
                    mybir.ActivationFunctionType.Identity,
                    bias=nmr[:, bch : bch + 1],
                    scale=rstd[:, bch : bch + 1],
                )
                nc.vector.tensor_mul(w_sb[:, bch], w_sb[:, bch], lsc_row)
                nc.vector.tensor_add(out_sb[:, bch], w_sb[:, bch], lbi_row)
                eng = nc.sync if bch == 0 else nc.scalar
                eng.dma_start(d_out[bch * P : (bch + 1) * P, :], out_sb[:, bch])

    nc.compile()
    return nc


def _get_nc():
    if "nc" not in _cache:
        _cache["nc"] = _build_nc()
    return _cache["nc"]


def make_in_maps(**inputs):
    """Shard + pre-transpose + pre-cast full inputs into 8 per-core input maps."""
    import ml_dtypes

    bf = ml_dtypes.bfloat16
    f32 = lambda x: np.ascontiguousarray(np.asarray(x), dtype=np.float32)
    h_A = f32(inputs["h_A"])
    alpha = f32(inputs["alpha"])
    pool = np.asarray(inputs["pool_vectors"], dtype=np.float32)
    W_base = f32(inputs["W_base"])
    b_base = f32(inputs["b_base"]).reshape(D_B)
    gamma = float(np.asarray(inputs["gamma"]).reshape(()))
    ln_scale = f32(inputs["ln_scale"]).reshape(D_A)
    ln_bias = f32(inputs["ln_bias"]).reshape(D_A)

    U = pool[:, : D_B * R].reshape(N, D_B, R)
    V = pool[:, D_B * R : D_B * R + R * D_A].reshape(N, R, D_A)
    bE = pool[:, D_B * R + R * D_A : D_B * R + R * D_A + D_B]

    # pool chunks in final SBUF layout, bf16, o-interleaved per partition row
    pool_pk = np.empty((P, 4, PO_W), bf)
    for o in range(4):
        nsl = slice(o * P, (o + 1) * P)
        # VT[p_a, ach, r, pn] = V[o*128+pn, r, ach*128+p_a]
        vt = V[nsl].transpose(2, 1, 0).reshape(2, P, R, P).transpose(1, 0, 2, 3)
        pool_pk[:, o, PO_VT : PO_VT + 1024] = vt.reshape(P, 1024).astype(bf)
        # U2[p_n, r, cch, pc] = U[o*128+p_n, cch*128+pc, r]
        u2 = U[nsl].transpose(0, 2, 1).reshape(P, R, 2, P)
        pool_pk[:, o, PO_U2 : PO_U2 + 1024] = u2.reshape(P, 1024).astype(bf)
        pool_pk[:, o, PO_BE : PO_BE + 256] = bE[nsl].astype(bf)

    ident = np.eye(P, dtype=np.float32).astype(bf)
    ge = np.empty((P, 2), np.float32)
    ge[:, 0] = gamma
    ge[:, 1] = LN_EPS
    wbt = np.ascontiguousarray(
        W_base.T.reshape(2, P, D_B).transpose(1, 0, 2).reshape(P, 512)
    ).astype(bf)

    in_maps = []
    for i in range(NC_COUNT):
        sl = slice(i * BS, (i + 1) * BS)
        sa = np.zeros((P, SA_W), bf)
        hat = h_A[sl].T.reshape(2, P, BS).transpose(1, 0, 2).reshape(P, 512)
        sa[:, SA_HAT : SA_HAT + 512] = hat.astype(bf)
        alt = alpha[sl].T.reshape(4, P, BS).transpose(1, 0, 2).reshape(P, 1024)
        sa[:, SA_ALT : SA_ALT + 1024] = alt.astype(bf)
        sa[:, SA_ID : SA_ID + P] = ident
        sa[:, SA_WBT : SA_WBT + 512] = wbt
        sa[:, SA_GE : SA_GE + 4] = ge.view(bf)

        sb = np.zeros((P, SB_W), bf)
        haf = np.ascontiguousarray(
            h_A[sl].reshape(2, P, D_A).transpose(1, 0, 2).reshape(P, 512)
        )
        sb[:, SB_HAF : SB_HAF + 1024] = haf.view(bf)
        sb[:, SB_LSC : SB_LSC + 256] = ln_scale.astype(bf)[None, :]
        sb[:, SB_LBI : SB_LBI + 256] = ln_bias.astype(bf)[None, :]
        sb[0, SB_BB : SB_BB + 256] = b_base.astype(bf)

        in_maps.append({"sma": sa, "pool": pool_pk, "smb": sb})
    return in_maps


def run_kernel(trace=False, **inputs):
    from concourse.bass_utils import run_bass_kernel_spmd

    nc = _get_nc()
    in_maps = make_in_maps(**inputs)
    res = run_bass_kernel_spmd(nc, in_maps, core_ids=list(range(NC_COUNT)), trace=trace)
    out = np.concatenate([r["out"] for r in res.results], axis=0)
    return out.astype(np.float32), res


def kernel(**inputs) -> np.ndarray:
    out, _ = run_kernel(trace=False, **inputs)
    return out


# revision 13
# speedup vs baseline: 1.1497x; 1.0936x over previous
"""Bass/Trainium2 kernel for nn_DWAMiddleLayer (low-rank MoE weight-assembly layer).

Math (reference):
    U    = pool[:, :1024].reshape(N, DB, R)      # [512, 256, 4]
    V    = pool[:, 1024:2048].reshape(N, R, DA)  # [512, 4, 256]
    bE   = pool[:, 2048:2304]                    # [512, 256]
    h_t  = h_A @ W_base.T
           + sum_r (alpha * (h_A @ V_r.T)) @ U_r          # never materialize W_assembled
           + alpha @ bE + b_base
    y    = h_A + gamma * h_t ; out = LayerNorm(y) * ln_scale + ln_bias

Distribution: data-parallel over batch B=2048 across 8 cores (BS=256 rows each);
pool/W_base/vectors replicated.

v4: the low-rank path (V, U, h_A, s) runs in fp8-e4m3 with DoubleRow matmuls
(2 k-tiles per instruction, 2x PE rate) and half the pool HBM bytes; power-of-2
scales keep everything in e4m3 range (V*64, U*32, alpha*2^-4 with bE*16
compensating; the fp8 accumulator carries 2^11 which the epilogue divides out).
The bias/base path stays bf16 in a second PSUM accumulator; both are merged by
the transpose matmuls. gamma-scaling of h_t makes the fp8 error negligible in
the output. Bulk data flows via SWDGE (gpsimd, ~340GB/s with >=4KB/partition
rows); sa via the sync HWDGE ring in parallel. PE warmed with dummy matmuls so
the HAM clock gate lifts before real work.
"""

import numpy as np

B, N, D_A, D_B, R = 2048, 512, 256, 256, 4
NC_COUNT = 8
BS = B // NC_COUNT  # 256 batch rows per core
P = 128
LN_EPS = 1e-5

V_SCALE = 64.0
U_SCALE = 32.0
A_SCALE = 1.0 / 16.0   # alpha^T pre-scale (bE * 16 compensates on the bias path)
ACC8_SCALE = 1.0 / (V_SCALE * U_SCALE * A_SCALE)  # psum carries 2^11

# ---- packed small tensor A (bf16 cols; fp8 regions bitcast), needed early ----
SA_HA8 = 0      # hA^T fp8  [p_a, 2 ach, 256 b]   (512 fp8 = 256 bf16 cols)
SA_HAT = 256    # hA^T bf16 [p_a, 2 ach, 256 b]   (base-mm rhs)
SA_ALT = 768    # alpha^T * 2^-4 bf16 [p_n, 4 och, 256 b]
SA_ID = 1792    # ident     [p, 128] bf16
SA_WBT = 1920   # W_base^T  [p_a, 2 ach, 256 c] bf16
SA_BE = 2432    # bE * 16   [p_n, 4 o, 256 c] bf16
SA_GE = 3456    # fp32 [gamma, eps] bitcast -> 4 bf16 cols
SA_W = 3460
# ---- packed small tensor B (bf16 cols), needed late (epilogue) ----
SB_HAF = 0      # h_A fp32  [p_b, 2 bch, 256 a] bitcast -> 1024 bf16 cols
SB_LSC = 1024   # ln_scale  [p, 256] replicated
SB_LBI = 1280   # ln_bias   [p, 256] replicated
SB_BB = 1536    # b_base    row0 only [1, 256]
SB_W = 1792
# ---- fp8 pool pair layout: d_p8 bf16 [128, 2 pair, 2048] -> fp8 [., ., 4096]
#      per o within pair: [VT 1024 | U2 1024] fp8 cols
P8_VT = 0       # [ach(2), r(4), pn(128)]
P8_U2 = 1024    # [r(4), cch(2), pc(128)]

N_WARM = 7  # warm-up matmuls (j=512): ~3us of cold PE activity to lift HAM

_cache = {}


def _build_nc():
    import concourse.mybir as mybir
    import concourse.tile as tile
    from concourse import bacc

    fp32 = mybir.dt.float32
    bf16 = mybir.dt.bfloat16
    fp8 = mybir.dt.float8e4
    DR = mybir.MatmulPerfMode.DoubleRow

    nc = bacc.Bacc("TRN2", target_bir_lowering=False)

    # ---- DRAM I/O (per-core shard shapes) ----
    d_sa = nc.dram_tensor("sma", [P, SA_W], bf16, kind="ExternalInput")
    d_p8 = nc.dram_tensor("p8", [P, 2, 2048], bf16, kind="ExternalInput")
    d_sb = nc.dram_tensor("smb", [P, SB_W], bf16, kind="ExternalInput")
    d_out = nc.dram_tensor("out", [BS, D_A], fp32, kind="ExternalOutput")

    with tile.TileContext(nc) as tc:
        with (
            tc.tile_pool(name="persist", bufs=1) as persist,
            tc.tile_pool(name="stage", bufs=2) as stage,
            tc.tile_pool(name="sm", bufs=3) as sm,
            tc.tile_pool(name="pp_t", bufs=2, space="PSUM") as pp_t,
            tc.tile_pool(name="pp_a8", bufs=1, space="PSUM") as pp_a8,
            tc.tile_pool(name="pp_ab", bufs=1, space="PSUM") as pp_ab,
            tc.tile_pool(name="pp_tr", bufs=1, space="PSUM") as pp_tr,
            tc.tile_pool(name="pp_w", bufs=1, space="PSUM") as pp_w,
        ):
            # ---------- PE warm-up: junk matmuls to lift the HAM clock gate ----------
            wsrc = persist.tile([P, 512], bf16)
            nc.vector.memset(wsrc, 0.0)
            ones_row = persist.tile([1, BS], bf16)
            nc.vector.memset(ones_row, 1.0)
            warm_ps = pp_w.tile([P, 512], fp32, tag="warm")
            for _ in range(N_WARM):
                nc.tensor.matmul(
                    warm_ps, lhsT=wsrc[:, 0:P], rhs=wsrc, start=True, stop=True,
                    skip_group_check=True,
                )

            # ---------- loads ----------
            # bulk fp8 pool pairs + late smalls via SWDGE (gpsimd, FIFO order);
            # sa via the sync HWDGE ring concurrently.
            p8t = [
                stage.tile([P, 2048], bf16, tag="p8", name=f"p8_{pr}")
                for pr in range(2)
            ]
            for pr in range(2):
                nc.gpsimd.dma_start(p8t[pr], d_p8[:, pr])
            sb = persist.tile([P, SB_W], bf16)
            nc.gpsimd.dma_start(sb, d_sb[:])
            sa = persist.tile([P, SA_W], bf16)
            nc.sync.dma_start(sa, d_sa[:])

            hA8 = sa[:, SA_HA8 : SA_HA8 + 256].bitcast(fp8).rearrange(
                "p (a b) -> p a b", a=2
            )
            hAT = sa[:, SA_HAT : SA_HAT + 512].rearrange("p (a b) -> p a b", a=2)
            alphaT = sa[:, SA_ALT : SA_ALT + 1024].rearrange("p (o b) -> p o b", o=4)
            ident_b = sa[:, SA_ID : SA_ID + P]
            WbT = sa[:, SA_WBT : SA_WBT + 512].rearrange("p (a c) -> p a c", a=2)
            bE16 = sa[:, SA_BE : SA_BE + 1024].rearrange("p (o c) -> p o c", o=4)
            ge = sa[:, SA_GE : SA_GE + 4].bitcast(fp32)
            gamma_col = ge[:, 0:1]
            eps_col = ge[:, 1:2]
            hA_f32 = sb[:, SB_HAF : SB_HAF + 1024].bitcast(fp32).rearrange(
                "p (o a) -> p o a", o=2
            )
            lsc_row = sb[:, SB_LSC : SB_LSC + 256]
            lbi_row = sb[:, SB_LBI : SB_LBI + 256]
            bb_row = sb[0:1, SB_BB : SB_BB + 256]

            # warm the ACT tables (Copy for the copies, Sqrt for the LN tail)
            warm_act = sm.tile([P, 1], fp32, tag="warmact")
            nc.scalar.activation(
                warm_act, wsrc[:, 0:1], mybir.ActivationFunctionType.Copy
            )
            nc.scalar.activation(
                warm_act, wsrc[:, 0:1], mybir.ActivationFunctionType.Sqrt
            )

            # ---------- h_t^T accumulators: fp8 path (x 2^11) + bf16 path ----------
            acc8 = pp_a8.tile([P, 2, BS], fp32, tag="a8")
            accb = pp_ab.tile([P, 2, BS], fp32, tag="ab")
            st8 = [False, False]
            stb = [False, False]

            def mm8(ch, lhsT, rhs, last=False):
                nc.tensor.matmul(
                    acc8[:, ch], lhsT=lhsT, rhs=rhs,
                    start=(not st8[ch]), stop=last,
                    perf_mode=DR, skip_group_check=True,
                )
                st8[ch] = True

            def mmb(ch, lhsT, rhs, last=False):
                nc.tensor.matmul(
                    accb[:, ch], lhsT=lhsT, rhs=rhs,
                    start=(not stb[ch]), stop=last,
                    skip_group_check=True,
                )
                stb[ch] = True

            # ---------- main pipeline over expert-chunk pairs ----------
            for pr in range(2):
                pc8 = p8t[pr].bitcast(fp8).rearrange("p (o f) -> p o f", o=2)
                s8 = sm.tile([P, 2, 4, BS], fp8, tag="s8")
                for oi in range(2):
                    o = pr * 2 + oi
                    VT_o = pc8[:, oi, P8_VT : P8_VT + 1024].rearrange(
                        "p (a r q) -> p a r q", a=2, r=4
                    )
                    # mm1 (DoubleRow, contraction a=256 in one matmul per r)
                    t_ps = pp_t.tile([P, 4, BS], fp32, tag="t")
                    for r in range(4):
                        nc.tensor.matmul(
                            t_ps[:, r],
                            lhsT=VT_o[:, :, r],
                            rhs=hA8,
                            start=True,
                            stop=True,
                            perf_mode=DR,
                        )
                    # s = (alpha * 2^-4) * t : direct-from-PSUM DVE multiply
                    nc.vector.tensor_mul(
                        s8[:, oi], t_ps,
                        alphaT[:, o : o + 1, :].to_broadcast((P, 4, BS)),
                    )
                # mm2 (DoubleRow over the o-pair, contraction n=256)
                U2_pr = pc8[:, :, P8_U2 : P8_U2 + 1024].rearrange(
                    "p o (r c q) -> p o r c q", r=4, c=2
                )
                for r in range(4):
                    for ch in range(2):
                        mm8(ch, U2_pr[:, :, r, ch], s8[:, :, r],
                            last=(pr == 1 and r == 3 and ch == 1))

            # bias-mm (bf16): h_t^T += (16*bE)^T @ (alpha^T * 2^-4)
            for o in range(4):
                for ch in range(2):
                    mmb(ch, bE16[:, o, ch * P : (ch + 1) * P], alphaT[:, o])
            # base-mm + b_base rank-1 (bf16)
            for ch in range(2):
                for ach in range(2):
                    mmb(ch, WbT[:, ach, ch * P : (ch + 1) * P], hAT[:, ach])
                mmb(ch, bb_row[:, ch * P : (ch + 1) * P], ones_row, last=True)

            # ---------- epilogue: merge paths in the transpose, LN ----------
            ht8 = sm.tile([P, 2, BS], bf16, tag="ht8")
            nc.scalar.activation(
                ht8, acc8, mybir.ActivationFunctionType.Copy, scale=ACC8_SCALE
            )
            htb = sm.tile([P, 2, BS], bf16, tag="htb")
            nc.scalar.activation(htb, accb, mybir.ActivationFunctionType.Copy)

            ht_ps = pp_tr.tile([P, 2, D_A], fp32, tag="tr")
            for bch in range(2):
                for cch in range(2):
                    for k, src in enumerate((ht8, htb)):
                        nc.tensor.matmul(
                            ht_ps[:, bch, cch * P : (cch + 1) * P],
                            lhsT=src[:, cch, bch * P : (bch + 1) * P],
                            rhs=ident_b,
                            start=(k == 0),
                            stop=(k == 1),
                            skip_group_check=True,
                        )

            # y = h_A + gamma * h_t (fp32 residual), both b-chunks in one pass
            y_sb = sm.tile([P, 2, D_A], fp32, tag="y")
            nc.vector.scalar_tensor_tensor(
                y_sb,
                in0=ht_ps,
                scalar=gamma_col,
                in1=hA_f32,
                op0=mybir.AluOpType.mult,
                op1=mybir.AluOpType.add,
            )
            stats = sm.tile([P, 2, 6], fp32, tag="st")
            mv = sm.tile([P, 2, 2], fp32, tag="mv")
            for bch in range(2):
                nc.vector.bn_stats(stats[:, bch], y_sb[:, bch])
                nc.vector.bn_aggr(mv[:, bch], stats[:, bch])
            # rstd = 1/sqrt(var + eps) for both chunks at once
            rstd = sm.tile([P, 2], fp32, tag="rstd")
            nc.scalar.activation(
                rstd,
                mv[:, :, 1],
                mybir.ActivationFunctionType.Sqrt,
                bias=eps_col,
            )
            nc.vector.reciprocal(rstd, rstd)
            # nmr = -mu * rstd (per-partition bias for the ACT normalize)
            nmr = sm.tile([P, 2], fp32, tag="nmr")
            nc.vector.scalar_tensor_tensor(
                nmr,
                in0=mv[:, :, 0],
                scalar=-1.0,
                in1=rstd,
                op0=mybir.AluOpType.mult,
                op1=mybir.AluOpType.mult,
            )
            # per-batch-chunk: normalize on ACT, scale/bias on DVE, DMA out
            w_sb = sm.tile([P, 2, D_A], fp32, tag="w")
            out_sb = sm.tile([P, 2, D_A], fp32, tag="out")
            for bch in range(2):
                nc.scalar.activation(
                    w_sb[:, bch],
                    y_sb[:, bch],
                    mybir.ActivationFunctionType.Identity,
                    bias=nmr[:, bch : bch + 1],
                    scale=rstd[:, bch : bch + 1],
                )
                nc.vector.tensor_mul(w_sb[:, bch], w_sb[:, bch], lsc_row)
                nc.vector.tensor_add(out_sb[:, bch], w_sb[:, bch], lbi_row)
                eng = nc.sync if bch == 0 else nc.scalar
                eng.dma_start(d_out[bch * P : (bch + 1) * P, :], out_sb[:, bch])

    nc.compile()
    return nc


def _get_nc():
    if "nc" not in _cache:
        _cache["nc"] = _build_nc()
    return _cache["nc"]


def make_in_maps(**inputs):
    """Shard + pre-transpose + pre-cast full inputs into 8 per-core input maps."""
    import ml_dtypes

    bf = ml_dtypes.bfloat16
    f8 = ml_dtypes.float8_e4m3fn
    f32 = lambda x: np.ascontiguousarray(np.asarray(x), dtype=np.float32)

    def to8(x):  # TRN e4m3 tops out at +-240 (vs OCP 448)
        return np.clip(x, -240.0, 240.0).astype(f8)

    h_A = f32(inputs["h_A"])
    alpha = f32(inputs["alpha"])
    pool = np.asarray(inputs["pool_vectors"], dtype=np.float32)
    W_base = f32(inputs["W_base"])
    b_base = f32(inputs["b_base"]).reshape(D_B)
    gamma = float(np.asarray(inputs["gamma"]).reshape(()))
    ln_scale = f32(inputs["ln_scale"]).reshape(D_A)
    ln_bias = f32(inputs["ln_bias"]).reshape(D_A)

    U = pool[:, : D_B * R].reshape(N, D_B, R)
    V = pool[:, D_B * R : D_B * R + R * D_A].reshape(N, R, D_A)
    bE = pool[:, D_B * R + R * D_A : D_B * R + R * D_A + D_B]

    # fp8 pool pairs: [p, pair, o_in_pair, [VT | U2]] packed as bf16 bit-carrier
    p8 = np.empty((P, 2, 2, 2048), f8)
    for o in range(4):
        nsl = slice(o * P, (o + 1) * P)
        # VT[p_a, ach, r, pn] = V[o*128+pn, r, ach*128+p_a] * V_SCALE
        vt = V[nsl].transpose(2, 1, 0).reshape(2, P, R, P).transpose(1, 0, 2, 3)
        p8[:, o // 2, o % 2, P8_VT : P8_VT + 1024] = to8(
            vt.reshape(P, 1024) * V_SCALE
        )
        # U2[p_n, r, cch, pc] = U[o*128+p_n, cch*128+pc, r] * U_SCALE
        u2 = U[nsl].transpose(0, 2, 1).reshape(P, R, 2, P)
        p8[:, o // 2, o % 2, P8_U2 : P8_U2 + 1024] = to8(
            u2.reshape(P, 1024) * U_SCALE
        )
    p8_carrier = p8.reshape(P, 2, 4096).view(np.uint8).view(np.uint16).view(bf)

    ident = np.eye(P, dtype=np.float32).astype(bf)
    ge = np.empty((P, 2), np.float32)
    ge[:, 0] = gamma
    ge[:, 1] = LN_EPS
    wbt = np.ascontiguousarray(
        W_base.T.reshape(2, P, D_B).transpose(1, 0, 2).reshape(P, 512)
    ).astype(bf)
    # bE16[p_n, o, c] = bE[o*128+p_n, c] * 16
    be16 = np.ascontiguousarray(
        (bE * 16.0).reshape(4, P, D_B).transpose(1, 0, 2).reshape(P, 1024)
    ).astype(bf)

    in_maps = []
    for i in range(NC_COUNT):
        sl = slice(i * BS, (i + 1) * BS)
        hat = h_A[sl].T.reshape(2, P, BS).transpose(1, 0, 2).reshape(P, 512)
        alt = alpha[sl].T.reshape(4, P, BS).transpose(1, 0, 2).reshape(P, 1024)

        sa = np.zeros((P, SA_W), bf)
        ha8 = to8(hat).view(np.uint8).view(np.uint16).view(bf)  # [P, 256]
        sa[:, SA_HA8 : SA_HA8 + 256] = ha8
        sa[:, SA_HAT : SA_HAT + 512] = hat.astype(bf)
        sa[:, SA_ALT : SA_ALT + 1024] = (alt * A_SCALE).astype(bf)
        sa[:, SA_ID : SA_ID + P] = ident
        sa[:, SA_WBT : SA_WBT + 512] = wbt
        sa[:, SA_BE : SA_BE + 1024] = be16
        sa[:, SA_GE : SA_GE + 4] = ge.view(bf)

        sb = np.zeros((P, SB_W), bf)
        haf = np.ascontiguousarray(
            h_A[sl].reshape(2, P, D_A).transpose(1, 0, 2).reshape(P, 512)
        )
        sb[:, SB_HAF : SB_HAF + 1024] = haf.view(bf)
        sb[:, SB_LSC : SB_LSC + 256] = ln_scale.astype(bf)[None, :]
        sb[:, SB_LBI : SB_LBI + 256] = ln_bias.astype(bf)[None, :]
        sb[0, SB_BB : SB_BB + 256] = b_base.astype(bf)

        in_maps.append({"sma": sa, "p8": p8_carrier, "smb": sb})
    return in_maps


def run_kernel(trace=False, **inputs):
    from concourse.bass_utils import run_bass_kernel_spmd

    nc = _get_nc()
    in_maps = make_in_maps(**inputs)
    res = run_bass_kernel_spmd(nc, in_maps, core_ids=list(range(NC_COUNT)), trace=trace)
    out = np.concatenate([r["out"] for r in res.results], axis=0)
    return out.astype(np.float32), res


def kernel(**inputs) -> np.ndarray:
    out, _ = run_kernel(trace=False, **inputs)
    return out


# revision 14
# speedup vs baseline: 1.1685x; 1.0164x over previous
"""Bass/Trainium2 kernel for nn_DWAMiddleLayer (low-rank MoE weight-assembly layer).

Math (reference):
    U    = pool[:, :1024].reshape(N, DB, R)      # [512, 256, 4]
    V    = pool[:, 1024:2048].reshape(N, R, DA)  # [512, 4, 256]
    bE   = pool[:, 2048:2304]                    # [512, 256]
    h_t  = h_A @ W_base.T
           + sum_r (alpha * (h_A @ V_r.T)) @ U_r          # never materialize W_assembled
           + alpha @ bE + b_base
    y    = h_A + gamma * h_t ; out = LayerNorm(y) * ln_scale + ln_bias

Distribution: data-parallel over batch B=2048 across 8 cores (BS=256 rows each);
pool/W_base/vectors replicated.

v5: the whole h_t matmul path runs in fp8-e4m3 DoubleRow (2 k-tiles per
instruction, 2x PE rate, half the pool HBM bytes) into ONE accumulator that
carries a 128x power-of-2 scale (V*64, U*32, alpha*2^-4; bE*128, W_base*128);
the epilogue divides it out during the PSUM->SBUF copy. gamma*b_base is folded
into the fp32 residual h_A on the host (exact), removing the rank-1 matmul.
gamma-scaling of h_t keeps the fp8 error ~1e-3 in the output. Bulk data flows
via SWDGE (gpsimd, ~340GB/s); sa via the sync HWDGE ring in parallel. The PE
is warmed with dummy matmuls until real data arrives so the HAM clock gate
(1.2 vs 2.4 GHz) stays lifted. LN epilogue is pipelined per batch-chunk across
Scalar (normalize via per-partition scale/bias) and Vector, with per-chunk
output DMAs on both HWDGE rings.
"""

import numpy as np

B, N, D_A, D_B, R = 2048, 512, 256, 256, 4
NC_COUNT = 8
BS = B // NC_COUNT  # 256 batch rows per core
P = 128
LN_EPS = 1e-5

V_SCALE = 64.0
U_SCALE = 32.0
A_SCALE = 1.0 / 16.0       # alpha^T pre-scale for the s-path
W_SCALE = 128.0            # W_base^T and bE fp8 scales (match the accumulator)
ACC_SCALE = V_SCALE * U_SCALE * A_SCALE  # = 128: acc8 carries 128 * h_t

# ---- packed small tensor A (bf16 cols; fp8 regions bitcast), needed early ----
SA_HA8 = 0      # hA^T fp8         [p_a, 2 ach, 256 b]  (256 carrier cols)
SA_ALT = 256    # alpha^T * 2^-4 bf16 [p_n, 4 och, 256 b]
SA_A8 = 1280    # alpha^T fp8      [p_n, 4 och, 256 b]  (512 carrier cols)
SA_ID = 1792    # ident            [p, 128] bf16
SA_WB8 = 1920   # W_base^T * 128 fp8 [p_a, 2 ach, 256 c] (256 carrier cols)
SA_BE8 = 2176   # bE * 128 fp8     [p_n, 4 o, 256 c]    (512 carrier cols)
SA_GE = 2688    # fp32 [gamma, eps] bitcast -> 4 bf16 cols
SA_W = 2692
# ---- packed small tensor B (bf16 cols), needed late (epilogue) ----
SB_HAF = 0      # (h_A + gamma*b_base) fp32 [p_b, 2 bch, 256 a] -> 1024 bf16 cols
SB_LSC = 1024   # ln_scale  [p, 256] replicated
SB_LBI = 1280   # ln_bias   [p, 256] replicated
SB_W = 1536
# ---- fp8 pool pair layout: d_p8 bf16 [128, 2 pair, 2048] -> fp8 [., ., 4096]
#      per o within pair: [VT 1024 | U2 1024] fp8 cols
P8_VT = 0       # [ach(2), r(4), pn(128)]
P8_U2 = 1024    # [r(4), cch(2), pc(128)]

N_WARM = 12  # warm-up matmuls (j=512): bridge PE activity until data arrives

_cache = {}


def _build_nc():
    import concourse.mybir as mybir
    import concourse.tile as tile
    from concourse import bacc

    fp32 = mybir.dt.float32
    bf16 = mybir.dt.bfloat16
    fp8 = mybir.dt.float8e4
    DR = mybir.MatmulPerfMode.DoubleRow

    nc = bacc.Bacc("TRN2", target_bir_lowering=False)

    # ---- DRAM I/O (per-core shard shapes) ----
    d_sa = nc.dram_tensor("sma", [P, SA_W], bf16, kind="ExternalInput")
    d_p8 = nc.dram_tensor("p8", [P, 2, 2048], bf16, kind="ExternalInput")
    d_sb = nc.dram_tensor("smb", [P, SB_W], bf16, kind="ExternalInput")
    d_out = nc.dram_tensor("out", [BS, D_A], fp32, kind="ExternalOutput")

    with tile.TileContext(nc) as tc:
        with (
            tc.tile_pool(name="persist", bufs=1) as persist,
            tc.tile_pool(name="stage", bufs=2) as stage,
            tc.tile_pool(name="sm", bufs=3) as sm,
            tc.tile_pool(name="pp_t", bufs=2, space="PSUM") as pp_t,
            tc.tile_pool(name="pp_a8", bufs=1, space="PSUM") as pp_a8,
            tc.tile_pool(name="pp_tr", bufs=1, space="PSUM") as pp_tr,
            tc.tile_pool(name="pp_w", bufs=1, space="PSUM") as pp_w,
        ):
            # ---------- PE warm-up: junk matmuls to lift the HAM clock gate ----------
            wsrc = persist.tile([P, 512], bf16)
            nc.vector.memset(wsrc, 0.0)
            warm_ps = pp_w.tile([P, 512], fp32, tag="warm")
            for _ in range(N_WARM):
                nc.tensor.matmul(
                    warm_ps, lhsT=wsrc[:, 0:P], rhs=wsrc, start=True, stop=True,
                    skip_group_check=True,
                )

            # ---------- loads ----------
            # bulk fp8 pool pairs + late smalls via SWDGE (gpsimd, FIFO order);
            # sa via the sync HWDGE ring concurrently.
            p8t = [
                stage.tile([P, 2048], bf16, tag="p8", name=f"p8_{pr}")
                for pr in range(2)
            ]
            for pr in range(2):
                nc.gpsimd.dma_start(p8t[pr], d_p8[:, pr])
            sb = persist.tile([P, SB_W], bf16)
            nc.gpsimd.dma_start(sb, d_sb[:])
            sa = persist.tile([P, SA_W], bf16)
            nc.sync.dma_start(sa, d_sa[:])

            hA8 = sa[:, SA_HA8 : SA_HA8 + 256].bitcast(fp8).rearrange(
                "p (a b) -> p a b", a=2
            )
            alphaT = sa[:, SA_ALT : SA_ALT + 1024].rearrange("p (o b) -> p o b", o=4)
            a8 = sa[:, SA_A8 : SA_A8 + 512].bitcast(fp8).rearrange(
                "p (o b) -> p o b", o=4
            )
            ident_b = sa[:, SA_ID : SA_ID + P]
            Wb8 = sa[:, SA_WB8 : SA_WB8 + 256].bitcast(fp8).rearrange(
                "p (a c) -> p a c", a=2
            )
            bE8 = sa[:, SA_BE8 : SA_BE8 + 512].bitcast(fp8).rearrange(
                "p (o c) -> p o c", o=4
            )
            ge = sa[:, SA_GE : SA_GE + 4].bitcast(fp32)
            gamma_col = ge[:, 0:1]
            eps_col = ge[:, 1:2]
            hA_f32 = sb[:, SB_HAF : SB_HAF + 1024].bitcast(fp32).rearrange(
                "p (o a) -> p o a", o=2
            )
            lsc_row = sb[:, SB_LSC : SB_LSC + 256]
            lbi_row = sb[:, SB_LBI : SB_LBI + 256]

            # warm the ACT tables (Copy for the copies, Sqrt for the LN tail)
            warm_act = sm.tile([P, 1], fp32, tag="warmact")
            nc.scalar.activation(
                warm_act, wsrc[:, 0:1], mybir.ActivationFunctionType.Copy
            )
            nc.scalar.activation(
                warm_act, wsrc[:, 0:1], mybir.ActivationFunctionType.Sqrt
            )

            # ---------- h_t^T accumulator (fp8 DoubleRow path, x128 scale) ----------
            acc8 = pp_a8.tile([P, 2, BS], fp32, tag="a8")
            st8 = [False, False]

            def mm8(ch, lhsT, rhs, last=False):
                nc.tensor.matmul(
                    acc8[:, ch], lhsT=lhsT, rhs=rhs,
                    start=(not st8[ch]), stop=last,
                    perf_mode=DR, skip_group_check=True,
                )
                st8[ch] = True

            # ---------- main pipeline over expert-chunk pairs ----------
            for pr in range(2):
                pc8 = p8t[pr].bitcast(fp8).rearrange("p (o f) -> p o f", o=2)
                s8 = sm.tile([P, 2, 4, BS], fp8, tag="s8")
                for oi in range(2):
                    o = pr * 2 + oi
                    VT_o = pc8[:, oi, P8_VT : P8_VT + 1024].rearrange(
                        "p (a r q) -> p a r q", a=2, r=4
                    )
                    # mm1 (DoubleRow, contraction a=256 in one matmul per r)
                    t_ps = pp_t.tile([P, 4, BS], fp32, tag="t")
                    for r in range(4):
                        nc.tensor.matmul(
                            t_ps[:, r],
                            lhsT=VT_o[:, :, r],
                            rhs=hA8,
                            start=True,
                            stop=True,
                            perf_mode=DR,
                        )
                    # s = (alpha * 2^-4) * t : direct-from-PSUM DVE multiply
                    nc.vector.tensor_mul(
                        s8[:, oi], t_ps,
                        alphaT[:, o : o + 1, :].to_broadcast((P, 4, BS)),
                    )
                # bias-mm (DoubleRow): 128*bE^T @ alpha^T, contraction n-pair
                for ch in range(2):
                    mm8(ch, bE8[:, 2 * pr : 2 * pr + 2, ch * P : (ch + 1) * P],
                        a8[:, 2 * pr : 2 * pr + 2])
                if pr == 0:
                    # base-mm (DoubleRow): 128*W_base^T @ hA^T, contraction a
                    for ch in range(2):
                        mm8(ch, Wb8[:, :, ch * P : (ch + 1) * P], hA8)
                # mm2 (DoubleRow over the o-pair, contraction n=256)
                U2_pr = pc8[:, :, P8_U2 : P8_U2 + 1024].rearrange(
                    "p o (r c q) -> p o r c q", r=4, c=2
                )
                for r in range(4):
                    for ch in range(2):
                        mm8(ch, U2_pr[:, :, r, ch], s8[:, :, r],
                            last=(pr == 1 and r == 3 and ch == 1))

            # ---------- epilogue: h_t back to batch-major, residual + LN ----------
            ht8 = sm.tile([P, 2, BS], bf16, tag="ht8")
            nc.scalar.activation(
                ht8, acc8, mybir.ActivationFunctionType.Copy, scale=1.0 / ACC_SCALE
            )
            ht_ps = pp_tr.tile([P, 2, D_A], fp32, tag="tr")
            for bch in range(2):
                for cch in range(2):
                    nc.tensor.matmul(
                        ht_ps[:, bch, cch * P : (cch + 1) * P],
                        lhsT=ht8[:, cch, bch * P : (bch + 1) * P],
                        rhs=ident_b,
                        start=True,
                        stop=True,
                        skip_group_check=True,
                    )

            # y = (h_A + gamma*b_base) + gamma * h_t', per-batch-chunk pipeline
            y_sb = sm.tile([P, 2, D_A], fp32, tag="y")
            stats = sm.tile([P, 2, 6], fp32, tag="st")
            mv = sm.tile([P, 2, 2], fp32, tag="mv")
            for bch in range(2):
                nc.vector.scalar_tensor_tensor(
                    y_sb[:, bch],
                    in0=ht_ps[:, bch],
                    scalar=gamma_col,
                    in1=hA_f32[:, bch],
                    op0=mybir.AluOpType.mult,
                    op1=mybir.AluOpType.add,
                )
                nc.vector.bn_stats(stats[:, bch], y_sb[:, bch])
                nc.vector.bn_aggr(mv[:, bch], stats[:, bch])
            # rstd = 1/sqrt(var + eps); nmr = -mu * rstd
            rstd = sm.tile([P, 2], fp32, tag="rstd")
            nc.scalar.activation(
                rstd,
                mv[:, :, 1],
                mybir.ActivationFunctionType.Sqrt,
                bias=eps_col,
            )
            nc.vector.reciprocal(rstd, rstd)
            nmr = sm.tile([P, 2], fp32, tag="nmr")
            nc.vector.scalar_tensor_tensor(
                nmr,
                in0=mv[:, :, 0],
                scalar=-1.0,
                in1=rstd,
                op0=mybir.AluOpType.mult,
                op1=mybir.AluOpType.mult,
            )
            # per-batch-chunk: normalize on ACT, scale/bias on DVE, DMA out
            w_sb = sm.tile([P, 2, D_A], fp32, tag="w")
            out_sb = sm.tile([P, 2, D_A], fp32, tag="out")
            for bch in range(2):
                nc.scalar.activation(
                    w_sb[:, bch],
                    y_sb[:, bch],
                    mybir.ActivationFunctionType.Identity,
                    bias=nmr[:, bch : bch + 1],
                    scale=rstd[:, bch : bch + 1],
                )
                nc.vector.tensor_mul(w_sb[:, bch], w_sb[:, bch], lsc_row)
                nc.vector.tensor_add(out_sb[:, bch], w_sb[:, bch], lbi_row)
                eng = nc.sync if bch == 0 else nc.scalar
                eng.dma_start(d_out[bch * P : (bch + 1) * P, :], out_sb[:, bch])

    nc.compile()
    return nc


def _get_nc():
    if "nc" not in _cache:
        _cache["nc"] = _build_nc()
    return _cache["nc"]


def make_in_maps(**inputs):
    """Shard + pre-transpose + pre-cast full inputs into 8 per-core input maps."""
    import ml_dtypes

    bf = ml_dtypes.bfloat16
    f8 = ml_dtypes.float8_e4m3fn
    f32 = lambda x: np.ascontiguousarray(np.asarray(x), dtype=np.float32)

    def to8c(x):  # fp8 bytes packed into a bf16 bit-carrier, 2 per column
        q = np.clip(x, -240.0, 240.0).astype(f8)  # TRN e4m3 tops out at +-240
        return q.reshape(q.shape[0], -1).view(np.uint8).view(np.uint16).view(bf)

    h_A = f32(inputs["h_A"])
    alpha = f32(inputs["alpha"])
    pool = np.asarray(inputs["pool_vectors"], dtype=np.float32)
    W_base = f32(inputs["W_base"])
    b_base = f32(inputs["b_base"]).reshape(D_B)
    gamma = float(np.asarray(inputs["gamma"]).reshape(()))
    ln_scale = f32(inputs["ln_scale"]).reshape(D_A)
    ln_bias = f32(inputs["ln_bias"]).reshape(D_A)

    U = pool[:, : D_B * R].reshape(N, D_B, R)
    V = pool[:, D_B * R : D_B * R + R * D_A].reshape(N, R, D_A)
    bE = pool[:, D_B * R + R * D_A : D_B * R + R * D_A + D_B]

    # fp8 pool pairs: [p, pair, o_in_pair, [VT | U2]] packed as bf16 bit-carrier
    p8 = np.empty((P, 2, 2, 2048), np.float32)
    for o in range(4):
        nsl = slice(o * P, (o + 1) * P)
        vt = V[nsl].transpose(2, 1, 0).reshape(2, P, R, P).transpose(1, 0, 2, 3)
        p8[:, o // 2, o % 2, P8_VT : P8_VT + 1024] = vt.reshape(P, 1024) * V_SCALE
        u2 = U[nsl].transpose(0, 2, 1).reshape(P, R, 2, P)
        p8[:, o // 2, o % 2, P8_U2 : P8_U2 + 1024] = u2.reshape(P, 1024) * U_SCALE
    p8_carrier = to8c(p8.reshape(P, -1)).reshape(P, 2, 2048)

    ident = np.eye(P, dtype=np.float32).astype(bf)
    ge = np.empty((P, 2), np.float32)
    ge[:, 0] = gamma
    ge[:, 1] = LN_EPS
    wbt = np.ascontiguousarray(
        W_base.T.reshape(2, P, D_B).transpose(1, 0, 2).reshape(P, 512)
    )
    be = np.ascontiguousarray(
        bE.reshape(4, P, D_B).transpose(1, 0, 2).reshape(P, 1024)
    )

    in_maps = []
    for i in range(NC_COUNT):
        sl = slice(i * BS, (i + 1) * BS)
        hat = h_A[sl].T.reshape(2, P, BS).transpose(1, 0, 2).reshape(P, 512)
        alt = alpha[sl].T.reshape(4, P, BS).transpose(1, 0, 2).reshape(P, 1024)

        sa = np.zeros((P, SA_W), bf)
        sa[:, SA_HA8 : SA_HA8 + 256] = to8c(hat)
        sa[:, SA_ALT : SA_ALT + 1024] = (alt * A_SCALE).astype(bf)
        sa[:, SA_A8 : SA_A8 + 512] = to8c(alt)
        sa[:, SA_ID : SA_ID + P] = ident
        sa[:, SA_WB8 : SA_WB8 + 256] = to8c(wbt * W_SCALE)
        sa[:, SA_BE8 : SA_BE8 + 512] = to8c(be * W_SCALE)
        sa[:, SA_GE : SA_GE + 4] = ge.view(bf)

        sb = np.zeros((P, SB_W), bf)
        # fold gamma*b_base into the residual (exact, host-side fp32)
        haf = np.ascontiguousarray(
            (h_A[sl] + gamma * b_base[None, :])
            .reshape(2, P, D_A).transpose(1, 0, 2).reshape(P, 512)
        )
        sb[:, SB_HAF : SB_HAF + 1024] = haf.view(bf)
        sb[:, SB_LSC : SB_LSC + 256] = ln_scale.astype(bf)[None, :]
        sb[:, SB_LBI : SB_LBI + 256] = ln_bias.astype(bf)[None, :]

        in_maps.append({"sma": sa, "p8": p8_carrier, "smb": sb})
    return in_maps


def run_kernel(trace=False, **inputs):
    from concourse.bass_utils import run_bass_kernel_spmd

    nc = _get_nc()
    in_maps = make_in_maps(**inputs)
    res = run_bass_kernel_spmd(nc, in_maps, core_ids=list(range(NC_COUNT)), trace=trace)
    out = np.concatenate([r["out"] for r in res.results], axis=0)
    return out.astype(np.float32), res


def kernel(**inputs) -> np.ndarray:
    out, _ = run_kernel(trace=False, **inputs)
    return out
